# revision 17
# baseline (speedup 1.0000x reference)
"""Trainium2 Bass kernel for nn_EnterpriseNeuralMemory (scatter_memory).

Sharding: data-parallel over batch — 8 batch elements, one per NeuronCore.
No collectives needed (router mean is per-batch-element and chunk pooling is
chunk-local).

Per-core algorithm (batch element b, all layouts transposed = [feature, pos]):
  logitsT = attn_w.T @ x.T          (PE, bf16, 16 pos-tiles of 512)
  E^T = exp(logitsT)                (ACT, PSUM->SBUF bf16)
  P^T = x^T * E^T                   (DVE bf16 2x)
  Z = segsum64(E^T), N = segsum64(P^T)  (DVE s1/s2 bf16 pair-adds,
                                         final f32 reduce on GpSimd/Pool)
  m = segsum64(x)/64 via block-ones matmul on PE (natural layout x)
  conv_pool  = W0@(m+u/64) + W1@m + W2@(m+v/64) + conv_b
               (boundary algebra: u/v from strided firsts/lasts columns)
  router: mean of chunk-first tokens -> 2-layer MLP -> softmax(3)
  out = r0*m + r1*(N/Z) + r2*conv_pool   (emitted in 3 chunk-ranges so the
                                          output DMA streams out early)
"""

import numpy as np
import ml_dtypes

BF16 = ml_dtypes.bfloat16

B, S, D = 8, 8192, 512
C = 64                      # chunk size
NCH = S // C                # 128 chunks
P = 128                     # partitions
DT = D // P                 # 4 feature tiles
JT = 512                    # positions per matmul tile
NJ = S // JT                # 16 pos-tiles
HID, NEXP = 128, 3

N_CORES = 8

_CACHE = {}


def _make_pools(ctx, tc):
    return {
        "consts": ctx.enter_context(tc.tile_pool(name="consts", bufs=1)),
        "xtp": ctx.enter_context(tc.tile_pool(name="xtp", bufs=4)),
        "xnp": ctx.enter_context(tc.tile_pool(name="xnp", bufs=2)),
        "bigp": ctx.enter_context(tc.tile_pool(name="bigp", bufs=1)),
        "grids": ctx.enter_context(tc.tile_pool(name="grids", bufs=1)),
        "scratch": ctx.enter_context(tc.tile_pool(name="scratch", bufs=1)),
        "ps_lg": ctx.enter_context(tc.tile_pool(name="ps_lg", bufs=5, space="PSUM")),
        "ps_m": ctx.enter_context(tc.tile_pool(name="ps_m", bufs=1, space="PSUM")),
        "ps_epi": ctx.enter_context(tc.tile_pool(name="ps_epi", bufs=2, space="PSUM")),
    }


def _emit_body(pools, nc, tc, dram, mybir):
    """Emit one full forward pass for one core."""
    f32 = mybir.dt.float32
    bf16 = mybir.dt.bfloat16
    AF = mybir.ActivationFunctionType
    OP = mybir.AluOpType

    consts = pools["consts"]
    xtp = pools["xtp"]
    xnp = pools["xnp"]
    bigp = pools["bigp"]
    grids = pools["grids"]
    scratch = pools["scratch"]
    ps_lg = pools["ps_lg"]
    ps_m = pools["ps_m"]
    ps_epi = pools["ps_epi"]

    # [512, X] dram tensors load as one [128, 4, X] tile each (one DMA).
    def load4(src, cols, dtype, nm):
        t = consts.tile([P, DT, cols], dtype, tag=nm, name=nm)
        nc.sync.dma_start(
            out=t[:], in_=src[:, :].rearrange("(a p) c -> p a c", p=P))
        return t

    # ---- head: minimal-latency first work --------------------------------
    # DMA order tuned so the first matmul can issue ~2us in: aw0 + the first
    # feature-tile of xt0 arrive first, then the rest streams while PE works.
    aw = []
    for k in range(DT):
        t = consts.tile([P, D], bf16, tag=f"aw{k}", name=f"aw{k}")
        aw.append(t)
    nc.sync.dma_start(out=aw[0][:], in_=dram["attn_w"][0:P, :])
    xt0 = xtp.tile([P, DT, JT], bf16, tag="xt", name="xt0")
    nc.sync.dma_start(out=xt0[:, 0:1], in_=dram["xT"][0:P, 0:JT].rearrange(
        "(a p) c -> p a c", p=P))
    nc.sync.dma_start(
        out=xt0[:, 1:DT],
        in_=dram["xT"][P:D, 0:JT].rearrange("(a p) c -> p a c", p=P))
    for k in range(1, DT):
        nc.sync.dma_start(out=aw[k][:], in_=dram["attn_w"][k * P:(k + 1) * P, :])
    # prefetch the next two stream tiles ahead of everything non-urgent so
    # PE never waits on the serialized DMA queue
    xt_pre = {}
    for j in (1, 2):
        t = xtp.tile([P, DT, JT], bf16, tag="xt", name=f"xt{j}")
        nc.sync.dma_start(
            out=t[:],
            in_=dram["xT"][:, j * JT:(j + 1) * JT].rearrange(
                "(a p) c -> p a c", p=P))
        xt_pre[j] = t
    G = consts.tile([P, 2 * P], bf16, tag="G", name="G")
    nc.sync.dma_start(out=G[:], in_=dram["G"][:])
    xn0 = xnp.tile([P, 4, D], bf16, tag="xn", name="xn0")
    nc.sync.dma_start(
        out=xn0[:], in_=dram["xn"][0:JT, :].rearrange("(t p) c -> p t c", p=P))

    # grids for segsum results (combined across d-tiles)
    Zc = grids.tile([P, DT, NCH], f32, tag="Zc", name="Zc")[:]
    Nc = grids.tile([P, DT, NCH], f32, tag="Nc", name="Nc")[:]
    m_ps = ps_m.tile([P, D], f32, tag="m_ps", name="m_ps")

    ones11 = consts.tile([1, 1], f32, tag="ones11", name="ones11")
    nc.vector.memset(ones11[:], 1.0)
    ones1p = consts.tile([1, P], f32, tag="ones1p", name="ones1p")
    nc.vector.memset(ones1p[:], 1.0)

    def emit_front():
        # front work that depends only on host-prepped firsts/lasts; DMAs
        # deferred into the stream so they don't delay the xt/xn tiles
        fp4 = load4(dram["fpad"], NCH + 1, f32, "fp4")
        lp4 = load4(dram["lpad"], NCH + 1, f32, "lp4")
        rw14 = load4(dram["router_w1"], HID, f32, "rw14")
        rw1 = [rw14[:, k] for k in range(DT)]
        rb1 = consts.tile([1, HID], f32, tag="rb1", name="rb1")
        nc.sync.dma_start(out=rb1[:], in_=dram["router_b1"][:])
        rw2 = consts.tile([HID, NEXP], f32, tag="rw2", name="rw2")
        nc.sync.dma_start(out=rw2[:], in_=dram["router_w2"][:])
        rb2 = consts.tile([1, NEXP], f32, tag="rb2", name="rb2")
        nc.sync.dma_start(out=rb2[:], in_=dram["router_b2"][:])

        # boundary terms on the (otherwise idle at this point) GpSimd engine
        u = grids.tile([P, DT, NCH], f32, tag="u", name="u")
        nc.gpsimd.tensor_tensor(out=u[:], in0=lp4[:, :, 0:NCH],
                                in1=lp4[:, :, 1:NCH + 1], op=OP.subtract)
        v = grids.tile([P, DT, NCH], f32, tag="v", name="v")
        nc.gpsimd.tensor_tensor(out=v[:], in0=fp4[:, :, 1:NCH + 1],
                                in1=fp4[:, :, 0:NCH], op=OP.subtract)
        xfs = grids.tile([P, DT], f32, tag="xfs", name="xfs")
        nc.vector.reduce_sum(out=xfs[:], in_=fp4[:, :, 0:NCH],
                             axis=mybir.AxisListType.X)

        # router MLP + softmax + broadcast of r (independent of the stream)
        xf = grids.tile([P, DT], f32, tag="xf", name="xf")
        nc.scalar.mul(xf[:], xfs[:], 1.0 / NCH)
        ps_h = ps_epi.tile([P, 1], f32, tag="epi", name="epi")
        for k in range(DT):
            nc.tensor.matmul(ps_h[:], rw1[k][:], xf[:, k:k + 1],
                             start=(k == 0), stop=False)
        nc.tensor.matmul(ps_h[:], rb1[:], ones11[:], start=False, stop=True)
        hsb = grids.tile([P, 1], f32, tag="hsb", name="hsb")
        nc.scalar.activation(out=hsb[:], in_=ps_h[:], func=AF.Relu)
        ps_r = ps_epi.tile([1, NEXP], f32, tag="epi", name="epi")
        nc.tensor.matmul(ps_r[:], hsb[:], rw2[:], start=True, stop=False)
        nc.tensor.matmul(ps_r[:], ones11[:], rb2[:], start=False, stop=True)
        rmax = grids.tile([1, 1], f32, tag="rmax", name="rmax")
        nc.vector.reduce_max(out=rmax[:], in_=ps_r[:],
                             axis=mybir.AxisListType.X)
        nrmax = grids.tile([1, 1], f32, tag="nrmax", name="nrmax")
        nc.vector.tensor_scalar_mul(nrmax[:], rmax[:], -1.0)
        er = grids.tile([1, NEXP], f32, tag="er", name="er")
        nc.scalar.activation(out=er[:], in_=ps_r[:], func=AF.Exp, bias=nrmax[:])
        rsum = grids.tile([1, 1], f32, tag="rsum", name="rsum")
        nc.vector.reduce_sum(out=rsum[:], in_=er[:], axis=mybir.AxisListType.X)
        rrec = grids.tile([1, 1], f32, tag="rrec", name="rrec")
        nc.vector.reciprocal(rrec[:], rsum[:])
        rvec = grids.tile([1, NEXP], f32, tag="rvec", name="rvec")
        nc.vector.tensor_scalar_mul(rvec[:], er[:], rrec[:])
        ps_b = ps_epi.tile([P, NEXP], f32, tag="epi", name="epi")
        nc.tensor.matmul(ps_b[:], ones1p[:], rvec[:], start=True, stop=True)
        rb = grids.tile([P, NEXP], f32, tag="rb", name="rb")
        nc.scalar.copy(rb[:], ps_b[:])
        return u, v, rb

    PIECES = [1, 1, 2, 2, 2, 2, 2, 2, 1, 1]  # small pieces: segsum work starts
    assert sum(PIECES) == NJ                 # early and ends with a tiny tail

    NCH_MAX = max(PIECES) * JT // C

    def make_tree_gen(Ep, Pp, ch0, nch):
        """Hybrid segsum per tensor (all 4 d-tiles in one op): two bf16
        pair-add levels on DVE, two f32 pair-add levels on GpSimd, then a
        cheap f32 reduce-of-4 on DVE. Splitting the tree across engines
        keeps DVE free for the stream-side multiplies."""
        for tile4, grid in ((Ep, Zc), (Pp, Nc)):
            view = tile4[:, :, 0:nch * C].rearrange("p a (n c) -> p a n c", c=C)
            s1 = scratch.tile([P, DT, NCH_MAX, C // 2], bf16, tag="s1",
                              name="s1", bufs=2)[:, :, 0:nch]
            nc.vector.tensor_tensor(
                out=s1, in0=view[:, :, :, 0:32], in1=view[:, :, :, 32:64],
                op=OP.add)
            yield
            s2 = scratch.tile([P, DT, NCH_MAX, C // 4], bf16, tag="s2",
                              name="s2", bufs=2)[:, :, 0:nch]
            nc.vector.tensor_tensor(
                out=s2, in0=s1[:, :, :, 0:16], in1=s1[:, :, :, 16:32],
                op=OP.add)
            yield
            s3 = scratch.tile([P, DT, NCH_MAX, C // 8], f32, tag="s3",
                              name="s3", bufs=2)[:, :, 0:nch]
            nc.gpsimd.tensor_tensor(
                out=s3, in0=s2[:, :, :, 0:8], in1=s2[:, :, :, 8:16],
                op=OP.add)
            yield
            s4 = scratch.tile([P, DT, NCH_MAX, C // 16], f32, tag="s4",
                              name="s4", bufs=2)[:, :, 0:nch]
            nc.gpsimd.tensor_tensor(
                out=s4, in0=s3[:, :, :, 0:4], in1=s3[:, :, :, 4:8],
                op=OP.add)
            yield
            nc.vector.reduce_sum(
                out=grid[:, :, ch0:ch0 + nch], in_=s4,
                axis=mybir.AxisListType.X)
            yield

    mconv_loads = {}

    def emit_mconv_load(step):
        # conv weights / bias / ident DMAs, one per stream tile so they
        # never delay an xt/xn tile in the serialized DMA queue
        if step < 3:
            w4 = load4(dram[f"w{step}T"], D, bf16, f"w{step}T4")
            mconv_loads[step] = [w4[:, k] for k in range(DT)]
        else:
            cbr = consts.tile([1, D], f32, tag="cbr", name="cbr")
            nc.sync.dma_start(out=cbr[:], in_=dram["conv_b_row"][:])
            ident = consts.tile([P, P], f32, tag="ident", name="ident")
            nc.sync.dma_start(out=ident[:], in_=dram["ident"][:])
            mconv_loads["cbr"] = cbr
            mconv_loads["ident"] = ident

    def emit_mconv():
        # ------- epilogue part A: m transpose + conv expert
        wT = mconv_loads
        cbr = mconv_loads["cbr"]
        ident = mconv_loads["ident"]
        u, v, rb = front_out

        # m: PSUM [128 chunks, 512 d] -> SBUF f32 (scaled 1/64) -> transpose
        m_nat = grids.tile([P, D], f32, tag="m_nat", name="m_nat")
        nc.scalar.mul(m_nat[:], m_ps[:], 1.0 / C)
        mT = grids.tile([P, DT, NCH], f32, tag="mT", name="mT")
        for k in range(DT):
            pst = ps_epi.tile([P, P], f32, tag="epi", name="epi")
            nc.tensor.transpose(pst[:], m_nat[:, k * P:(k + 1) * P], ident[:])
            nc.scalar.copy(mT[:, k], pst[:])
        mTb = grids.tile([P, DT, NCH], bf16, tag="mTb", name="mTb")
        nc.scalar.copy(mTb[:], mT[:])

        # a = m + u/64, c = m + v/64  (bf16 for matmul; on GpSimd)
        aTb = grids.tile([P, DT, NCH], bf16, tag="aTb", name="aTb")
        nc.vector.scalar_tensor_tensor(
            out=aTb[:], in0=u[:], scalar=1.0 / C, in1=mT[:],
            op0=OP.mult, op1=OP.add)
        cTb = grids.tile([P, DT, NCH], bf16, tag="cTb", name="cTb")
        nc.vector.scalar_tensor_tensor(
            out=cTb[:], in0=v[:], scalar=1.0 / C, in1=mT[:],
            op0=OP.mult, op1=OP.add)

        # conv expert: 12 matmuls + bias matmul, then copy to SBUF
        convT = grids.tile([P, DT, NCH], f32, tag="convT", name="convT")
        for o in range(DT):
            ps = ps_epi.tile([P, NCH], f32, tag="epi", name="epi")
            first = True
            for w, rhs4 in ((0, aTb), (1, mTb), (2, cTb)):
                for k in range(DT):
                    nc.tensor.matmul(
                        ps[:], wT[w][k][:, o * P:(o + 1) * P], rhs4[:, k],
                        start=first, stop=False)
                    first = False
            nc.tensor.matmul(
                ps[:], cbr[:, o * P:(o + 1) * P], ones1p[:],
                start=False, stop=True)
            nc.scalar.copy(convT[:, o], ps[:])

        # r0 * m term of the mix (ready early; ACT per-partition scale)
        tmp = grids.tile([P, DT, NCH], f32, tag="tmp", name="tmp")
        nc.scalar.mul(tmp[:], mT[:], rb[:, 0:1])
        return convT, mT, tmp

    rz = grids.tile([P, DT, NCH], f32, tag="rz", name="rz")
    attnT = grids.tile([P, DT, NCH], f32, tag="attnT", name="attnT")
    acc = grids.tile([P, DT, NCH], f32, tag="acc", name="acc")
    y4 = grids.tile([P, DT, NCH], f32, tag="y4", name="y4")

    def emit_mix(c0, c1):
        """Mix experts and DMA out chunks [c0:c1) (grids must be final).
        Reciprocal is DVE-only; the rest runs on GpSimd to keep DVE free
        for the stream-side multiplies and trees."""
        convT, mT, tmp = mconv_out
        u, v, rb = front_out
        sl = slice(c0, c1)
        nc.vector.reciprocal(rz[:, :, sl], Zc[:, :, sl])
        nc.gpsimd.tensor_tensor(out=attnT[:, :, sl], in0=Nc[:, :, sl],
                                in1=rz[:, :, sl], op=OP.mult)
        nc.vector.scalar_tensor_tensor(
            out=acc[:, :, sl], in0=attnT[:, :, sl], scalar=rb[:, 1:2],
            in1=tmp[:, :, sl], op0=OP.mult, op1=OP.add)
        nc.vector.scalar_tensor_tensor(
            out=y4[:, :, sl], in0=convT[:, :, sl], scalar=rb[:, 2:3],
            in1=acc[:, :, sl], op0=OP.mult, op1=OP.add)
        nc.sync.dma_start(
            out=dram["y"][:, :].rearrange("(a p) n -> p a n", p=P)[:, :, sl],
            in_=y4[:, :, sl])

    # ---------------- main streaming phase ----------------
    # xn (natural layout) and the chunk-sum matmuls for m start at j==2 so
    # PE's in-order stream never waits on the xn DMAs during the head; they
    # still finish by j==12, in time for the conv epilogue at j==13.
    pending = None
    jbase = 0
    jn = 0
    xn_cur = xn0
    xn_nxt = None
    xn_tile_idx = 0
    front_out = None
    mconv_out = None

    def gmm_until(limit):
        nonlocal jn, xn_cur, xn_nxt, xn_tile_idx
        while jn < min(limit, 4 * NJ):
            t = jn // 4
            if t != xn_tile_idx:
                xn_cur = xn_nxt
                xn_tile_idx = t
                if t + 1 < NJ:
                    xn_nxt = prefetch_xn(t + 1)
            nc.tensor.matmul(
                m_ps[:], G[:, P - 2 * jn:2 * P - 2 * jn], xn_cur[:, jn % 4],
                start=(jn == 0), stop=(jn == 4 * NJ - 1),
                skip_group_check=True)
            jn += 1

    def prefetch_xn(t):
        tile = xnp.tile([P, 4, D], bf16, tag="xn", name="xn")
        nc.sync.dma_start(
            out=tile[:],
            in_=dram["xn"][t * JT:(t + 1) * JT, :].rearrange(
                "(t p) c -> p t c", p=P))
        return tile

    def fetch_xt(j):
        if j in xt_pre:
            return xt_pre.pop(j)
        tile = xtp.tile([P, DT, JT], bf16, tag="xt", name="xt")
        nc.sync.dma_start(
            out=tile[:],
            in_=dram["xT"][:, j * JT:(j + 1) * JT].rearrange(
                "(a p) c -> p a c", p=P))
        return tile

    def last_tile_otile(xt, Ep, Pp, o, ps):
        """j==15: per-otile exp + multiply + segsum so the drain pipelines
        behind each PSUM bank instead of waiting for the whole tile."""
        nc.scalar.activation(out=Ep[:, o, 0:JT], in_=ps[:], func=AF.Exp)
        nc.vector.tensor_tensor(out=Pp[:, o, 0:JT], in0=xt[:, o],
                                in1=Ep[:, o, 0:JT], op=OP.mult)
        ch0 = 15 * JT // C
        for tpfx, src, grid in (("e", Ep, Zc), ("p", Pp, Nc)):
            view = src[:, o, 0:JT].rearrange("p (n c) -> p n c", c=C)
            # E-side adds go on GpSimd so DVE only carries the P-side chain
            # in the drain (they run concurrently per otile)
            eng = nc.gpsimd if tpfx == "e" else nc.vector
            l1 = scratch.tile([P, JT // C, C // 2], bf16, tag=f"l1{tpfx}{o}",
                              name=f"l1{tpfx}{o}")
            eng.tensor_tensor(out=l1[:], in0=view[:, :, 0:32],
                              in1=view[:, :, 32:64], op=OP.add)
            l2 = scratch.tile([P, JT // C, C // 4], bf16, tag=f"l2{tpfx}{o}",
                              name=f"l2{tpfx}{o}")
            eng.tensor_tensor(out=l2[:], in0=l1[:, :, 0:16],
                              in1=l1[:, :, 16:32], op=OP.add)
            l3 = scratch.tile([P, JT // C, C // 8], f32, tag=f"l3{tpfx}{o}",
                              name=f"l3{tpfx}{o}")
            nc.gpsimd.tensor_tensor(out=l3[:], in0=l2[:, :, 0:8],
                                    in1=l2[:, :, 8:16], op=OP.add)
            l4 = scratch.tile([P, JT // C, C // 16], f32, tag=f"l4{tpfx}{o}",
                              name=f"l4{tpfx}{o}")
            nc.gpsimd.tensor_tensor(out=l4[:], in0=l3[:, :, 0:4],
                                    in1=l3[:, :, 4:8], op=OP.add)
            nc.vector.reduce_sum(
                out=grid[:, o, ch0:ch0 + JT // C],
                in_=l4[:], axis=mybir.AxisListType.X)

    for pc in range(len(PIECES)):
        PJp = PIECES[pc]
        Ep = bigp.tile([P, DT, max(PIECES) * JT], bf16, tag="Ep", name="Ep",
                       bufs=2)
        Pp = bigp.tile([P, DT, max(PIECES) * JT], bf16, tag="Pp", name="Pp",
                       bufs=2)
        for jj in range(PJp):
            j = jbase + jj
            off = jj * JT
            xt = xt0 if j == 0 else fetch_xt(j)
            if j + 2 < NJ and j + 2 > 2:
                xt_pre[j + 2] = fetch_xt(j + 2)
            if j == 1:
                xn_nxt = prefetch_xn(1)
            if j == NJ - 1 and pending is not None:
                # drain the previous piece fully BEFORE the last tile's
                # per-otile trees so its reduce isn't stuck behind them in
                # DVE's in-order queue (the final mix needs it)
                for _ in pending:
                    pass
                pending = None
            if j == 0:
                # k-outer order so the first matmul only needs aw0 + the
                # first feature-tile of xt0 (shortest DMA critical path).
                ps4 = [ps_lg.tile([P, JT], f32, tag="lg", name="lg")
                       for _ in range(DT)]
                for k in range(DT):
                    for o in range(DT):
                        nc.tensor.matmul(
                            ps4[o][:], aw[k][:, o * P:(o + 1) * P], xt[:, k],
                            start=(k == 0), stop=(k == DT - 1))
                for o in range(DT):
                    nc.scalar.activation(
                        out=Ep[:, o, off:off + JT], in_=ps4[o][:], func=AF.Exp)
                nc.vector.tensor_tensor(
                    out=Pp[:, :, off:off + JT], in0=xt[:],
                    in1=Ep[:, :, off:off + JT], op=OP.mult)
            else:
                for o in range(DT):
                    ps = ps_lg.tile([P, JT], f32, tag="lg", name="lg")
                    for k in range(DT):
                        nc.tensor.matmul(
                            ps[:], aw[k][:, o * P:(o + 1) * P], xt[:, k],
                            start=(k == 0), stop=(k == DT - 1))
                    if j == NJ - 1:
                        last_tile_otile(xt, Ep, Pp, o, ps)
                    else:
                        nc.scalar.activation(
                            out=Ep[:, o, off:off + JT], in_=ps[:], func=AF.Exp)
                if j != NJ - 1:
                    nc.vector.tensor_tensor(
                        out=Pp[:, :, off:off + JT], in0=xt[:],
                        in1=Ep[:, :, off:off + JT], op=OP.mult)
            if j >= 2:
                gmm_until(6 * (j - 1))
            # interleave previous piece's segsum ops (5 per pos-tile)
            if pending is not None:
                for _ in range(5):
                    if next(pending, "done") == "done":
                        pending = None
                        break
            if j == 4:
                front_out = emit_front()
            if 7 <= j <= 10:
                emit_mconv_load(j - 7)
            if j == 13:
                mconv_out = emit_mconv()
            if j == 14:
                emit_mix(0, 64)       # pieces 0-4 drained long ago
            if j == 15:
                emit_mix(64, 112)     # piece 7 drained during piece 8
        if pending is not None:
            for _ in pending:
                pass
        if pc < len(PIECES) - 1:
            pending = make_tree_gen(Ep, Pp, jbase * JT // C, PJp * JT // C)
        jbase += PJp

    # ------- final mix (last piece's per-otile trees already emitted)
    emit_mix(112, NCH)


def _build(loop_iters=None):
    import concourse.bass as bass
    from concourse import bacc
    import concourse.mybir as mybir
    import concourse.tile as tile

    f32 = mybir.dt.float32
    bf16 = mybir.dt.bfloat16

    nc = bacc.Bacc(None, target_bir_lowering=False)
    dram = {
        "xT": nc.dram_tensor("xT", [D, S], bf16, kind="ExternalInput"),
        "xn": nc.dram_tensor("xn", [S, D], bf16, kind="ExternalInput"),
        "attn_w": nc.dram_tensor("attn_w", [D, D], bf16, kind="ExternalInput"),
        "w0T": nc.dram_tensor("w0T", [D, D], bf16, kind="ExternalInput"),
        "w1T": nc.dram_tensor("w1T", [D, D], bf16, kind="ExternalInput"),
        "w2T": nc.dram_tensor("w2T", [D, D], bf16, kind="ExternalInput"),
        "fpad": nc.dram_tensor("fpad", [D, NCH + 1], f32, kind="ExternalInput"),
        "lpad": nc.dram_tensor("lpad", [D, NCH + 1], f32, kind="ExternalInput"),
        "router_w1": nc.dram_tensor("router_w1", [D, HID], f32, kind="ExternalInput"),
        "router_b1": nc.dram_tensor("router_b1", [1, HID], f32, kind="ExternalInput"),
        "router_w2": nc.dram_tensor("router_w2", [HID, NEXP], f32, kind="ExternalInput"),
        "router_b2": nc.dram_tensor("router_b2", [1, NEXP], f32, kind="ExternalInput"),
        "conv_b_row": nc.dram_tensor("conv_b_row", [1, D], f32, kind="ExternalInput"),
        "G": nc.dram_tensor("G", [P, 2 * P], bf16, kind="ExternalInput"),
        "ident": nc.dram_tensor("ident", [P, P], f32, kind="ExternalInput"),
        "y": nc.dram_tensor("y", [D, NCH], f32, kind="ExternalOutput"),
    }
    from contextlib import ExitStack
    with tile.TileContext(nc) as tc:
        with ExitStack() as ctx:
            pools = _make_pools(ctx, tc)
            # Preload the activation-function table once, outside the loop
            # body (otherwise LoadActFuncSet costs ~1.3us on every iteration).
            warm = pools["consts"].tile([1, 1], mybir.dt.float32, tag="warm",
                                        name="warm")
            nc.vector.memset(warm[:], 0.0)
            warm2 = pools["consts"].tile([1, 1], mybir.dt.float32, tag="warm2",
                                         name="warm2")
            nc.scalar.activation(out=warm2[:], in_=warm[:],
                                 func=mybir.ActivationFunctionType.Exp)
            if loop_iters is None:
                _emit_body(pools, nc, tc, dram, mybir)
            else:
                ET = mybir.EngineType
                with tc.For_i(0, loop_iters, 1,
                              hint_engines=(ET.PE, ET.DVE, ET.Activation,
                                            ET.SP, ET.Pool)):
                    _emit_body(pools, nc, tc, dram, mybir)
    nc.finalize()
    return nc


def _host_prep(inputs):
    """Build per-core input maps from full inputs."""
    x = np.asarray(inputs["x"], dtype=np.float32)
    attn_w = np.asarray(inputs["attn_w"], dtype=np.float32)
    conv_w = np.asarray(inputs["conv_w"], dtype=np.float32)
    conv_b = np.asarray(inputs["conv_b"], dtype=np.float32)
    rw1 = np.asarray(inputs["router_w1"], dtype=np.float32)
    rb1 = np.asarray(inputs["router_b1"], dtype=np.float32)
    rw2 = np.asarray(inputs["router_w2"], dtype=np.float32)
    rb2 = np.asarray(inputs["router_b2"], dtype=np.float32)

    aw_bf = np.ascontiguousarray(attn_w).astype(BF16)
    w0T = np.ascontiguousarray(conv_w[:, :, 0].T).astype(BF16)
    w1T = np.ascontiguousarray(conv_w[:, :, 1].T).astype(BF16)
    w2T = np.ascontiguousarray(conv_w[:, :, 2].T).astype(BF16)
    G = np.zeros((P, 2 * P), BF16)
    G[0:C, P] = 1.0
    G[C:P, P + 1] = 1.0
    ident = np.eye(P, dtype=np.float32)
    rb1_2d = rb1.reshape(1, HID)
    rb2_2d = rb2.reshape(1, NEXP)
    cb_row = conv_b.reshape(1, D)

    in_maps = []
    for b in range(B):
        xb = x[b]
        F = xb[0::C]            # [NCH, D]
        L = xb[C - 1::C]
        fpad = np.zeros((D, NCH + 1), np.float32)
        fpad[:, 0:NCH] = F.T
        lpad = np.zeros((D, NCH + 1), np.float32)
        lpad[:, 1:NCH + 1] = L.T
        in_maps.append({
            "xT": np.ascontiguousarray(xb.T).astype(BF16),
            "xn": xb.astype(BF16),
            "attn_w": aw_bf,
            "w0T": w0T, "w1T": w1T, "w2T": w2T,
            "fpad": fpad, "lpad": lpad,
            "router_w1": rw1, "router_b1": rb1_2d,
            "router_w2": rw2, "router_b2": rb2_2d,
            "conv_b_row": cb_row, "G": G, "ident": ident,
        })
    return in_maps


def kernel(**inputs):
    from concourse.bass_utils import run_bass_kernel_spmd

    if "nc" not in _CACHE:
        _CACHE["nc"] = _build()
    nc = _CACHE["nc"]
    in_maps = _host_prep(inputs)
    res = run_bass_kernel_spmd(nc, in_maps, list(range(N_CORES)))
    out = np.stack([np.ascontiguousarray(res.results[b]["y"].T)
                    for b in range(B)])
    return out.astype(np.float32)


if __name__ == "__main__":
    rng = np.random.default_rng(0)
    fake = {
        "x": rng.standard_normal((B, S, D), dtype=np.float32),
        "attn_w": rng.standard_normal((D, D), dtype=np.float32) / np.sqrt(D),
        "attn_b": np.zeros(D, np.float32),
        "conv_w": rng.standard_normal((D, D, 3), dtype=np.float32) / np.sqrt(3 * D),
        "conv_b": np.zeros(D, np.float32),
        "router_w1": rng.standard_normal((D, HID), dtype=np.float32) / np.sqrt(D),
        "router_b1": np.zeros(HID, np.float32),
        "router_w2": rng.standard_normal((HID, NEXP), dtype=np.float32) / np.sqrt(HID),
        "router_b2": np.zeros(NEXP, np.float32),
    }
    y = kernel(**fake)
    print("kernel out", y.shape, y.dtype, np.abs(y).max())


# revision 29
# speedup vs baseline: 6.5683x; 6.5683x over previous
"""Trainium2 Bass kernel for nn_EnterpriseNeuralMemory (scatter_memory).

Sharding: data-parallel over batch — 8 batch elements, one per NeuronCore.
No collectives needed (router mean is per-batch-element and chunk pooling is
chunk-local).

Per-core algorithm (batch element b, all layouts transposed = [feature, pos]):
  logitsT = attn_w.T @ x.T          (PE, bf16, 16 pos-tiles of 512)
  E^T = exp(logitsT)                (ACT, PSUM->SBUF bf16)
  P^T = x^T * E^T                   (DVE bf16 2x)
  Z = segsum64(E^T), N = segsum64(P^T)  (DVE s1/s2 bf16 pair-adds,
                                         final f32 reduce on GpSimd/Pool)
  m = segsum64(x)/64 via block-ones matmul on PE (natural layout x)
  conv_pool  = W0@(m+u/64) + W1@m + W2@(m+v/64) + conv_b
               (boundary algebra: u/v from strided firsts/lasts columns)
  router: mean of chunk-first tokens -> 2-layer MLP -> softmax(3)
  out = r0*m + r1*(N/Z) + r2*conv_pool   (emitted in 3 chunk-ranges so the
                                          output DMA streams out early)
"""

import numpy as np
import ml_dtypes

BF16 = ml_dtypes.bfloat16

B, S, D = 8, 8192, 512
C = 64                      # chunk size
NCH = S // C                # 128 chunks
P = 128                     # partitions
DT = D // P                 # 4 feature tiles
JT = 512                    # positions per matmul tile
NJ = S // JT                # 16 pos-tiles
HID, NEXP = 128, 3

N_CORES = 8

_CACHE = {}


def _make_pools(ctx, tc):
    return {
        "consts": ctx.enter_context(tc.tile_pool(name="consts", bufs=1)),
        "xtp": ctx.enter_context(tc.tile_pool(name="xtp", bufs=4)),
        "xnp": ctx.enter_context(tc.tile_pool(name="xnp", bufs=2)),
        "bigp": ctx.enter_context(tc.tile_pool(name="bigp", bufs=1)),
        "grids": ctx.enter_context(tc.tile_pool(name="grids", bufs=1)),
        "scratch": ctx.enter_context(tc.tile_pool(name="scratch", bufs=1)),
        "ps_lg": ctx.enter_context(tc.tile_pool(name="ps_lg", bufs=5, space="PSUM")),
        "ps_m": ctx.enter_context(tc.tile_pool(name="ps_m", bufs=1, space="PSUM")),
        "ps_epi": ctx.enter_context(tc.tile_pool(name="ps_epi", bufs=2, space="PSUM")),
    }


def _emit_body(pools, nc, tc, dram, mybir):
    """Emit one full forward pass for one core."""
    f32 = mybir.dt.float32
    bf16 = mybir.dt.bfloat16
    AF = mybir.ActivationFunctionType
    OP = mybir.AluOpType

    consts = pools["consts"]
    xtp = pools["xtp"]
    xnp = pools["xnp"]
    bigp = pools["bigp"]
    grids = pools["grids"]
    scratch = pools["scratch"]
    ps_lg = pools["ps_lg"]
    ps_m = pools["ps_m"]
    ps_epi = pools["ps_epi"]

    # [512, X] dram tensors load as one [128, 4, X] tile each (one DMA).
    def load4(src, cols, dtype, nm):
        t = consts.tile([P, DT, cols], dtype, tag=nm, name=nm)
        nc.sync.dma_start(
            out=t[:], in_=src[:, :].rearrange("(a p) c -> p a c", p=P))
        return t

    # ---- head: minimal-latency first work --------------------------------
    # DMA order tuned so the first matmul can issue ~2us in: aw0 + the first
    # feature-tile of xt0 arrive first, then the rest streams while PE works.
    aw = []
    for k in range(DT):
        t = consts.tile([P, D], bf16, tag=f"aw{k}", name=f"aw{k}")
        aw.append(t)
    nc.sync.dma_start(out=aw[0][:], in_=dram["attn_w"][0:P, :])
    xt0 = xtp.tile([P, DT, JT], bf16, tag="xt", name="xt0")
    nc.sync.dma_start(out=xt0[:, 0:1], in_=dram["xT"][0:P, 0:JT].rearrange(
        "(a p) c -> p a c", p=P))
    nc.sync.dma_start(
        out=xt0[:, 1:DT],
        in_=dram["xT"][P:D, 0:JT].rearrange("(a p) c -> p a c", p=P))
    for k in range(1, DT):
        nc.sync.dma_start(out=aw[k][:], in_=dram["attn_w"][k * P:(k + 1) * P, :])
    # prefetch the next two stream tiles ahead of everything non-urgent so
    # PE never waits on the serialized DMA queue
    xt_pre = {}
    for j in (1, 2):
        t = xtp.tile([P, DT, JT], bf16, tag="xt", name=f"xt{j}")
        nc.sync.dma_start(
            out=t[:],
            in_=dram["xT"][:, j * JT:(j + 1) * JT].rearrange(
                "(a p) c -> p a c", p=P))
        xt_pre[j] = t
    G = consts.tile([P, 2 * P], bf16, tag="G", name="G")
    nc.sync.dma_start(out=G[:], in_=dram["G"][:])
    xn0 = xnp.tile([P, 4, D], bf16, tag="xn", name="xn0")
    nc.sync.dma_start(
        out=xn0[:], in_=dram["xn"][0:JT, :].rearrange("(t p) c -> p t c", p=P))

    # grids for segsum results (combined across d-tiles)
    Zc = grids.tile([P, DT, NCH], f32, tag="Zc", name="Zc")[:]
    Nc = grids.tile([P, DT, NCH], f32, tag="Nc", name="Nc")[:]
    m_ps = ps_m.tile([P, D], f32, tag="m_ps", name="m_ps")

    ones11 = consts.tile([1, 1], f32, tag="ones11", name="ones11")
    nc.vector.memset(ones11[:], 1.0)
    ones1p = consts.tile([1, P], f32, tag="ones1p", name="ones1p")
    nc.vector.memset(ones1p[:], 1.0)

    def emit_front():
        # front work that depends only on host-prepped firsts/lasts; DMAs
        # deferred into the stream so they don't delay the xt/xn tiles
        fp4 = load4(dram["fpad"], NCH + 1, f32, "fp4")
        lp4 = load4(dram["lpad"], NCH + 1, f32, "lp4")
        rw14 = load4(dram["router_w1"], HID, f32, "rw14")
        rw1 = [rw14[:, k] for k in range(DT)]
        rb1 = consts.tile([1, HID], f32, tag="rb1", name="rb1")
        nc.sync.dma_start(out=rb1[:], in_=dram["router_b1"][:])
        rw2 = consts.tile([HID, NEXP], f32, tag="rw2", name="rw2")
        nc.sync.dma_start(out=rw2[:], in_=dram["router_w2"][:])
        rb2 = consts.tile([1, NEXP], f32, tag="rb2", name="rb2")
        nc.sync.dma_start(out=rb2[:], in_=dram["router_b2"][:])

        # boundary terms on the (otherwise idle at this point) GpSimd engine
        u = grids.tile([P, DT, NCH], f32, tag="u", name="u")
        nc.vector.tensor_tensor(out=u[:], in0=lp4[:, :, 0:NCH],
                                in1=lp4[:, :, 1:NCH + 1], op=OP.subtract)
        v = grids.tile([P, DT, NCH], f32, tag="v", name="v")
        nc.vector.tensor_tensor(out=v[:], in0=fp4[:, :, 1:NCH + 1],
                                in1=fp4[:, :, 0:NCH], op=OP.subtract)
        xfs = grids.tile([P, DT], f32, tag="xfs", name="xfs")
        nc.vector.reduce_sum(out=xfs[:], in_=fp4[:, :, 0:NCH],
                             axis=mybir.AxisListType.X)

        # router MLP + softmax + broadcast of r (independent of the stream)
        xf = grids.tile([P, DT], f32, tag="xf", name="xf")
        nc.scalar.mul(xf[:], xfs[:], 1.0 / NCH)
        ps_h = ps_epi.tile([P, 1], f32, tag="epi", name="epi")
        for k in range(DT):
            nc.tensor.matmul(ps_h[:], rw1[k][:], xf[:, k:k + 1],
                             start=(k == 0), stop=False)
        nc.tensor.matmul(ps_h[:], rb1[:], ones11[:], start=False, stop=True)
        hsb = grids.tile([P, 1], f32, tag="hsb", name="hsb")
        nc.scalar.activation(out=hsb[:], in_=ps_h[:], func=AF.Relu)
        ps_r = ps_epi.tile([1, NEXP], f32, tag="epi", name="epi")
        nc.tensor.matmul(ps_r[:], hsb[:], rw2[:], start=True, stop=False)
        nc.tensor.matmul(ps_r[:], ones11[:], rb2[:], start=False, stop=True)
        rmax = grids.tile([1, 1], f32, tag="rmax", name="rmax")
        nc.vector.reduce_max(out=rmax[:], in_=ps_r[:],
                             axis=mybir.AxisListType.X)
        nrmax = grids.tile([1, 1], f32, tag="nrmax", name="nrmax")
        nc.vector.tensor_scalar_mul(nrmax[:], rmax[:], -1.0)
        er = grids.tile([1, NEXP], f32, tag="er", name="er")
        nc.scalar.activation(out=er[:], in_=ps_r[:], func=AF.Exp, bias=nrmax[:])
        rsum = grids.tile([1, 1], f32, tag="rsum", name="rsum")
        nc.vector.reduce_sum(out=rsum[:], in_=er[:], axis=mybir.AxisListType.X)
        rrec = grids.tile([1, 1], f32, tag="rrec", name="rrec")
        nc.vector.reciprocal(rrec[:], rsum[:])
        rvec = grids.tile([1, NEXP], f32, tag="rvec", name="rvec")
        nc.vector.tensor_scalar_mul(rvec[:], er[:], rrec[:])
        ps_b = ps_epi.tile([P, NEXP], f32, tag="epi", name="epi")
        nc.tensor.matmul(ps_b[:], ones1p[:], rvec[:], start=True, stop=True)
        rb = grids.tile([P, NEXP], f32, tag="rb", name="rb")
        nc.scalar.copy(rb[:], ps_b[:])
        return u, v, rb

    PIECES = [1, 1, 2, 2, 2, 2, 2, 2, 1, 1]  # small pieces: segsum work starts
    assert sum(PIECES) == NJ                 # early and ends with a tiny tail

    NCH_MAX = max(PIECES) * JT // C

    def make_tree_gen(Ep, Pp, ch0, nch):
        """Segsum per tensor (all 4 d-tiles in one op): four bf16 pair-add
        levels (DVE 2x mode) then a cheap fp32 reduce of the last 4 — the
        f32 reduce gets no DVE fast mode, so keep it small."""
        for tile4, grid in ((Ep, Zc), (Pp, Nc)):
            view = tile4[:, :, 0:nch * C].rearrange("p a (n c) -> p a n c", c=C)
            s1 = scratch.tile([P, DT, NCH_MAX, C // 2], bf16, tag="s1",
                              name="s1", bufs=2)[:, :, 0:nch]
            nc.vector.tensor_tensor(
                out=s1, in0=view[:, :, :, 0:32], in1=view[:, :, :, 32:64],
                op=OP.add)
            yield
            s2 = scratch.tile([P, DT, NCH_MAX, C // 4], bf16, tag="s2",
                              name="s2", bufs=2)[:, :, 0:nch]
            nc.vector.tensor_tensor(
                out=s2, in0=s1[:, :, :, 0:16], in1=s1[:, :, :, 16:32],
                op=OP.add)
            yield
            s3 = scratch.tile([P, DT, NCH_MAX, C // 8], bf16, tag="s3",
                              name="s3", bufs=2)[:, :, 0:nch]
            nc.vector.tensor_tensor(
                out=s3, in0=s2[:, :, :, 0:8], in1=s2[:, :, :, 8:16],
                op=OP.add)
            yield
            s4 = scratch.tile([P, DT, NCH_MAX, C // 16], bf16, tag="s4",
                              name="s4", bufs=2)[:, :, 0:nch]
            nc.vector.tensor_tensor(
                out=s4, in0=s3[:, :, :, 0:4], in1=s3[:, :, :, 4:8],
                op=OP.add)
            yield
            nc.vector.reduce_sum(
                out=grid[:, :, ch0:ch0 + nch], in_=s4,
                axis=mybir.AxisListType.X)
            yield
            if tile4 is Ep:
                # 1/Z for this piece now, in stream slack, so the output
                # mixes at the tail are only two ops per range
                nc.vector.reciprocal(rz[:, :, ch0:ch0 + nch],
                                     Zc[:, :, ch0:ch0 + nch])
                yield

    mconv_loads = {}

    def emit_mconv_load(step):
        # conv weights / bias / ident DMAs, one per stream tile so they
        # never delay an xt/xn tile in the serialized DMA queue
        if step < 3:
            w4 = load4(dram[f"w{step}T"], D, bf16, f"w{step}T4")
            mconv_loads[step] = [w4[:, k] for k in range(DT)]
        else:
            cbr = consts.tile([1, D], f32, tag="cbr", name="cbr")
            nc.sync.dma_start(out=cbr[:], in_=dram["conv_b_row"][:])
            ident = consts.tile([P, P], f32, tag="ident", name="ident")
            nc.sync.dma_start(out=ident[:], in_=dram["ident"][:])
            mconv_loads["cbr"] = cbr
            mconv_loads["ident"] = ident

    def emit_mconv():
        # ------- epilogue part A: m transpose + conv expert
        wT = mconv_loads
        cbr = mconv_loads["cbr"]
        ident = mconv_loads["ident"]
        u, v, rb = front_out

        # m: PSUM [128 chunks, 512 d] -> SBUF f32 (scaled 1/64) -> transpose
        m_nat = grids.tile([P, D], f32, tag="m_nat", name="m_nat")
        nc.scalar.mul(m_nat[:], m_ps[:], 1.0 / C)
        mT = grids.tile([P, DT, NCH], f32, tag="mT", name="mT")
        for k in range(DT):
            pst = ps_epi.tile([P, P], f32, tag="epi", name="epi")
            nc.tensor.transpose(pst[:], m_nat[:, k * P:(k + 1) * P], ident[:])
            nc.scalar.copy(mT[:, k], pst[:])
        mTb = grids.tile([P, DT, NCH], bf16, tag="mTb", name="mTb")
        nc.scalar.copy(mTb[:], mT[:])

        # a = m + u/64, c = m + v/64  (bf16 for matmul; on GpSimd)
        aTb = grids.tile([P, DT, NCH], bf16, tag="aTb", name="aTb")
        nc.vector.scalar_tensor_tensor(
            out=aTb[:], in0=u[:], scalar=1.0 / C, in1=mT[:],
            op0=OP.mult, op1=OP.add)
        cTb = grids.tile([P, DT, NCH], bf16, tag="cTb", name="cTb")
        nc.vector.scalar_tensor_tensor(
            out=cTb[:], in0=v[:], scalar=1.0 / C, in1=mT[:],
            op0=OP.mult, op1=OP.add)

        # conv expert: 12 matmuls + bias matmul, then copy to SBUF
        convT = grids.tile([P, DT, NCH], f32, tag="convT", name="convT")
        for o in range(DT):
            ps = ps_epi.tile([P, NCH], f32, tag="epi", name="epi")
            first = True
            for w, rhs4 in ((0, aTb), (1, mTb), (2, cTb)):
                for k in range(DT):
                    nc.tensor.matmul(
                        ps[:], wT[w][k][:, o * P:(o + 1) * P], rhs4[:, k],
                        start=first, stop=False)
                    first = False
            nc.tensor.matmul(
                ps[:], cbr[:, o * P:(o + 1) * P], ones1p[:],
                start=False, stop=True)
            nc.scalar.copy(convT[:, o], ps[:])

        # r0 * m term of the mix (ready early; ACT per-partition scale),
        # then fold in r2*conv so each output range needs only 2 DVE ops
        tmp = grids.tile([P, DT, NCH], f32, tag="tmp", name="tmp")
        nc.scalar.mul(tmp[:], mT[:], rb[:, 0:1])
        W = grids.tile([P, DT, NCH], f32, tag="W", name="W")
        nc.vector.scalar_tensor_tensor(
            out=W[:], in0=convT[:], scalar=rb[:, 2:3], in1=tmp[:],
            op0=OP.mult, op1=OP.add)
        return W

    rz = grids.tile([P, DT, NCH], f32, tag="rz", name="rz")
    attnT = grids.tile([P, DT, NCH], f32, tag="attnT", name="attnT")
    y4 = grids.tile([P, DT, NCH], f32, tag="y4", name="y4")

    def emit_mix(c0, c1):
        """Mix experts and DMA out chunks [c0:c1) (grids + rz final):
        attn = N*(1/Z), y = r1*attn + (r0*m + r2*conv), then stream out."""
        W = mconv_out
        u, v, rb = front_out
        sl = slice(c0, c1)
        nc.vector.tensor_tensor(out=attnT[:, :, sl], in0=Nc[:, :, sl],
                                in1=rz[:, :, sl], op=OP.mult)
        nc.vector.scalar_tensor_tensor(
            out=y4[:, :, sl], in0=attnT[:, :, sl], scalar=rb[:, 1:2],
            in1=W[:, :, sl], op0=OP.mult, op1=OP.add)
        nc.sync.dma_start(
            out=dram["y"][:, :].rearrange("(a p) n -> p a n", p=P)[:, :, sl],
            in_=y4[:, :, sl])

    # ---------------- main streaming phase ----------------
    # xn (natural layout) and the chunk-sum matmuls for m start at j==2 so
    # PE's in-order stream never waits on the xn DMAs during the head; they
    # still finish by j==12, in time for the conv epilogue at j==13.
    pending = None
    jbase = 0
    jn = 0
    xn_cur = xn0
    xn_nxt = None
    xn_tile_idx = 0
    front_out = None
    mconv_out = None

    def gmm_until(limit):
        nonlocal jn, xn_cur, xn_nxt, xn_tile_idx
        while jn < min(limit, 4 * NJ):
            t = jn // 4
            if t != xn_tile_idx:
                xn_cur = xn_nxt
                xn_tile_idx = t
                if t + 1 < NJ:
                    xn_nxt = prefetch_xn(t + 1)
            nc.tensor.matmul(
                m_ps[:], G[:, P - 2 * jn:2 * P - 2 * jn], xn_cur[:, jn % 4],
                start=(jn == 0), stop=(jn == 4 * NJ - 1),
                skip_group_check=True)
            jn += 1

    def prefetch_xn(t):
        tile = xnp.tile([P, 4, D], bf16, tag="xn", name="xn")
        nc.sync.dma_start(
            out=tile[:],
            in_=dram["xn"][t * JT:(t + 1) * JT, :].rearrange(
                "(t p) c -> p t c", p=P))
        return tile

    def fetch_xt(j):
        if j in xt_pre:
            return xt_pre.pop(j)
        tile = xtp.tile([P, DT, JT], bf16, tag="xt", name="xt")
        nc.sync.dma_start(
            out=tile[:],
            in_=dram["xT"][:, j * JT:(j + 1) * JT].rearrange(
                "(a p) c -> p a c", p=P))
        return tile

    def last_tile_otile(xt, Ep, Pp, o, ps):
        """j==15: per-otile exp + multiply + segsum so the drain pipelines
        behind each PSUM bank instead of waiting for the whole tile."""
        nc.scalar.activation(out=Ep[:, o, 0:JT], in_=ps[:], func=AF.Exp)
        nc.vector.tensor_tensor(out=Pp[:, o, 0:JT], in0=xt[:, o],
                                in1=Ep[:, o, 0:JT], op=OP.mult)
        ch0 = 15 * JT // C
        for tpfx, src, grid in (("e", Ep, Zc), ("p", Pp, Nc)):
            view = src[:, o, 0:JT].rearrange("p (n c) -> p n c", c=C)
            l1 = scratch.tile([P, JT // C, C // 2], bf16, tag=f"l1{tpfx}{o}",
                              name=f"l1{tpfx}{o}")
            nc.vector.tensor_tensor(out=l1[:], in0=view[:, :, 0:32],
                                    in1=view[:, :, 32:64], op=OP.add)
            l2 = scratch.tile([P, JT // C, C // 4], bf16, tag=f"l2{tpfx}{o}",
                              name=f"l2{tpfx}{o}")
            nc.vector.tensor_tensor(out=l2[:], in0=l1[:, :, 0:16],
                                    in1=l1[:, :, 16:32], op=OP.add)
            nc.vector.reduce_sum(
                out=grid[:, o, ch0:ch0 + JT // C],
                in_=l2[:], axis=mybir.AxisListType.X)
            if tpfx == "e":
                nc.vector.reciprocal(rz[:, o, ch0:ch0 + JT // C],
                                     Zc[:, o, ch0:ch0 + JT // C])

    for pc in range(len(PIECES)):
        PJp = PIECES[pc]
        Ep = bigp.tile([P, DT, max(PIECES) * JT], bf16, tag="Ep", name="Ep",
                       bufs=2)
        Pp = bigp.tile([P, DT, max(PIECES) * JT], bf16, tag="Pp", name="Pp",
                       bufs=2)
        for jj in range(PJp):
            j = jbase + jj
            off = jj * JT
            xt = xt0 if j == 0 else fetch_xt(j)
            if j + 2 < NJ and j + 2 > 2:
                xt_pre[j + 2] = fetch_xt(j + 2)
            if j == 1:
                xn_nxt = prefetch_xn(1)
            if j == NJ - 1 and pending is not None:
                # drain the previous piece fully BEFORE the last tile's
                # per-otile trees so its reduce isn't stuck behind them in
                # DVE's in-order queue (the final mix needs it)
                for _ in pending:
                    pass
                pending = None
            if j == 0:
                # k-outer order so the first matmul only needs aw0 + the
                # first feature-tile of xt0 (shortest DMA critical path).
                ps4 = [ps_lg.tile([P, JT], f32, tag="lg", name="lg")
                       for _ in range(DT)]
                for k in range(DT):
                    for o in range(DT):
                        nc.tensor.matmul(
                            ps4[o][:], aw[k][:, o * P:(o + 1) * P], xt[:, k],
                            start=(k == 0), stop=(k == DT - 1))
                for o in range(DT):
                    nc.scalar.activation(
                        out=Ep[:, o, off:off + JT], in_=ps4[o][:], func=AF.Exp)
                nc.vector.tensor_tensor(
                    out=Pp[:, :, off:off + JT], in0=xt[:],
                    in1=Ep[:, :, off:off + JT], op=OP.mult)
            else:
                for o in range(DT):
                    ps = ps_lg.tile([P, JT], f32, tag="lg", name="lg")
                    for k in range(DT):
                        nc.tensor.matmul(
                            ps[:], aw[k][:, o * P:(o + 1) * P], xt[:, k],
                            start=(k == 0), stop=(k == DT - 1))
                    if j == NJ - 1:
                        last_tile_otile(xt, Ep, Pp, o, ps)
                    else:
                        nc.scalar.activation(
                            out=Ep[:, o, off:off + JT], in_=ps[:], func=AF.Exp)
                if j != NJ - 1:
                    nc.vector.tensor_tensor(
                        out=Pp[:, :, off:off + JT], in0=xt[:],
                        in1=Ep[:, :, off:off + JT], op=OP.mult)
            if j >= 2:
                gmm_until(6 * (j - 1))
            # interleave previous piece's segsum ops (6 per pos-tile)
            if pending is not None:
                for _ in range(6):
                    if next(pending, "done") == "done":
                        pending = None
                        break
            if j == 4:
                front_out = emit_front()
            if 7 <= j <= 10:
                emit_mconv_load(j - 7)
            if j == 13:
                mconv_out = emit_mconv()
            if j == 14:
                emit_mix(0, 96)       # pieces 0-6 drained long ago
            if j == 15:
                emit_mix(96, 112)     # piece 7 drained by early j15
        if pending is not None:
            for _ in pending:
                pass
        if pc < len(PIECES) - 1:
            pending = make_tree_gen(Ep, Pp, jbase * JT // C, PJp * JT // C)
        jbase += PJp

    # ------- final mix (last piece's per-otile trees already emitted)
    emit_mix(112, NCH)


def _build(loop_iters=None):
    import concourse.bass as bass
    from concourse import bacc
    import concourse.mybir as mybir
    import concourse.tile as tile

    f32 = mybir.dt.float32
    bf16 = mybir.dt.bfloat16

    nc = bacc.Bacc(None, target_bir_lowering=False)
    dram = {
        "xT": nc.dram_tensor("xT", [D, S], bf16, kind="ExternalInput"),
        "xn": nc.dram_tensor("xn", [S, D], bf16, kind="ExternalInput"),
        "attn_w": nc.dram_tensor("attn_w", [D, D], bf16, kind="ExternalInput"),
        "w0T": nc.dram_tensor("w0T", [D, D], bf16, kind="ExternalInput"),
        "w1T": nc.dram_tensor("w1T", [D, D], bf16, kind="ExternalInput"),
        "w2T": nc.dram_tensor("w2T", [D, D], bf16, kind="ExternalInput"),
        "fpad": nc.dram_tensor("fpad", [D, NCH + 1], f32, kind="ExternalInput"),
        "lpad": nc.dram_tensor("lpad", [D, NCH + 1], f32, kind="ExternalInput"),
        "router_w1": nc.dram_tensor("router_w1", [D, HID], f32, kind="ExternalInput"),
        "router_b1": nc.dram_tensor("router_b1", [1, HID], f32, kind="ExternalInput"),
        "router_w2": nc.dram_tensor("router_w2", [HID, NEXP], f32, kind="ExternalInput"),
        "router_b2": nc.dram_tensor("router_b2", [1, NEXP], f32, kind="ExternalInput"),
        "conv_b_row": nc.dram_tensor("conv_b_row", [1, D], f32, kind="ExternalInput"),
        "G": nc.dram_tensor("G", [P, 2 * P], bf16, kind="ExternalInput"),
        "ident": nc.dram_tensor("ident", [P, P], f32, kind="ExternalInput"),
        "y": nc.dram_tensor("y", [D, NCH], f32, kind="ExternalOutput"),
    }
    from contextlib import ExitStack
    with tile.TileContext(nc) as tc:
        with ExitStack() as ctx:
            pools = _make_pools(ctx, tc)
            # Preload the activation-function table once, outside the loop
            # body (otherwise LoadActFuncSet costs ~1.3us on every iteration).
            warm = pools["consts"].tile([1, 1], mybir.dt.float32, tag="warm",
                                        name="warm")
            nc.vector.memset(warm[:], 0.0)
            warm2 = pools["consts"].tile([1, 1], mybir.dt.float32, tag="warm2",
                                         name="warm2")
            nc.scalar.activation(out=warm2[:], in_=warm[:],
                                 func=mybir.ActivationFunctionType.Exp)
            if loop_iters is None:
                _emit_body(pools, nc, tc, dram, mybir)
            else:
                ET = mybir.EngineType
                with tc.For_i(0, loop_iters, 1,
                              hint_engines=(ET.PE, ET.DVE, ET.Activation,
                                            ET.SP)):
                    _emit_body(pools, nc, tc, dram, mybir)
    nc.finalize()
    return nc


def _host_prep(inputs):
    """Build per-core input maps from full inputs."""
    x = np.asarray(inputs["x"], dtype=np.float32)
    attn_w = np.asarray(inputs["attn_w"], dtype=np.float32)
    conv_w = np.asarray(inputs["conv_w"], dtype=np.float32)
    conv_b = np.asarray(inputs["conv_b"], dtype=np.float32)
    rw1 = np.asarray(inputs["router_w1"], dtype=np.float32)
    rb1 = np.asarray(inputs["router_b1"], dtype=np.float32)
    rw2 = np.asarray(inputs["router_w2"], dtype=np.float32)
    rb2 = np.asarray(inputs["router_b2"], dtype=np.float32)

    aw_bf = np.ascontiguousarray(attn_w).astype(BF16)
    w0T = np.ascontiguousarray(conv_w[:, :, 0].T).astype(BF16)
    w1T = np.ascontiguousarray(conv_w[:, :, 1].T).astype(BF16)
    w2T = np.ascontiguousarray(conv_w[:, :, 2].T).astype(BF16)
    G = np.zeros((P, 2 * P), BF16)
    G[0:C, P] = 1.0
    G[C:P, P + 1] = 1.0
    ident = np.eye(P, dtype=np.float32)
    rb1_2d = rb1.reshape(1, HID)
    rb2_2d = rb2.reshape(1, NEXP)
    cb_row = conv_b.reshape(1, D)

    in_maps = []
    for b in range(B):
        xb = x[b]
        F = xb[0::C]            # [NCH, D]
        L = xb[C - 1::C]
        fpad = np.zeros((D, NCH + 1), np.float32)
        fpad[:, 0:NCH] = F.T
        lpad = np.zeros((D, NCH + 1), np.float32)
        lpad[:, 1:NCH + 1] = L.T
        in_maps.append({
            "xT": np.ascontiguousarray(xb.T).astype(BF16),
            "xn": xb.astype(BF16),
            "attn_w": aw_bf,
            "w0T": w0T, "w1T": w1T, "w2T": w2T,
            "fpad": fpad, "lpad": lpad,
            "router_w1": rw1, "router_b1": rb1_2d,
            "router_w2": rw2, "router_b2": rb2_2d,
            "conv_b_row": cb_row, "G": G, "ident": ident,
        })
    return in_maps


def kernel(**inputs):
    from concourse.bass_utils import run_bass_kernel_spmd

    if "nc" not in _CACHE:
        _CACHE["nc"] = _build()
    nc = _CACHE["nc"]
    in_maps = _host_prep(inputs)
    res = run_bass_kernel_spmd(nc, in_maps, list(range(N_CORES)))
    out = np.stack([np.ascontiguousarray(res.results[b]["y"].T)
                    for b in range(B)])
    return out.astype(np.float32)


if __name__ == "__main__":
    rng = np.random.default_rng(0)
    fake = {
        "x": rng.standard_normal((B, S, D), dtype=np.float32),
        "attn_w": rng.standard_normal((D, D), dtype=np.float32) / np.sqrt(D),
        "attn_b": np.zeros(D, np.float32),
        "conv_w": rng.standard_normal((D, D, 3), dtype=np.float32) / np.sqrt(3 * D),
        "conv_b": np.zeros(D, np.float32),
        "router_w1": rng.standard_normal((D, HID), dtype=np.float32) / np.sqrt(D),
        "router_b1": np.zeros(HID, np.float32),
        "router_w2": rng.standard_normal((HID, NEXP), dtype=np.float32) / np.sqrt(HID),
        "router_b2": np.zeros(NEXP, np.float32),
    }
    y = kernel(**fake)
    print("kernel out", y.shape, y.dtype, np.abs(y).max())


# revision 30
# speedup vs baseline: 218.0878x; 33.2031x over previous
"""Trainium2 Bass kernel for nn_EnterpriseNeuralMemory (scatter_memory).

Sharding: data-parallel over batch — 8 batch elements, one per NeuronCore.
No collectives needed (router mean is per-batch-element and chunk pooling is
chunk-local).

Per-core algorithm (batch element b, all layouts transposed = [feature, pos]):
  logitsT = attn_w.T @ x.T          (PE, bf16, 16 pos-tiles of 512)
  E^T = exp(logitsT)                (ACT, PSUM->SBUF bf16)
  P^T = x^T * E^T                   (DVE bf16 2x)
  Z = segsum64(E^T), N = segsum64(P^T)  (DVE s1/s2 bf16 pair-adds,
                                         final f32 reduce on GpSimd/Pool)
  m = segsum64(x)/64 via block-ones matmul on PE (natural layout x)
  conv_pool  = W0@(m+u/64) + W1@m + W2@(m+v/64) + conv_b
               (boundary algebra: u/v from strided firsts/lasts columns)
  router: mean of chunk-first tokens -> 2-layer MLP -> softmax(3)
  out = r0*m + r1*(N/Z) + r2*conv_pool   (emitted in 3 chunk-ranges so the
                                          output DMA streams out early)
"""

import numpy as np
import ml_dtypes

BF16 = ml_dtypes.bfloat16

B, S, D = 8, 8192, 512
C = 64                      # chunk size
NCH = S // C                # 128 chunks
P = 128                     # partitions
DT = D // P                 # 4 feature tiles
JT = 512                    # positions per matmul tile
NJ = S // JT                # 16 pos-tiles
HID, NEXP = 128, 3

N_CORES = 8

_CACHE = {}


def _make_pools(ctx, tc):
    return {
        "consts": ctx.enter_context(tc.tile_pool(name="consts", bufs=1)),
        "xtp": ctx.enter_context(tc.tile_pool(name="xtp", bufs=4)),
        "xnp": ctx.enter_context(tc.tile_pool(name="xnp", bufs=2)),
        "bigp": ctx.enter_context(tc.tile_pool(name="bigp", bufs=1)),
        "grids": ctx.enter_context(tc.tile_pool(name="grids", bufs=1)),
        "scratch": ctx.enter_context(tc.tile_pool(name="scratch", bufs=1)),
        "ps_lg": ctx.enter_context(tc.tile_pool(name="ps_lg", bufs=5, space="PSUM")),
        "ps_m": ctx.enter_context(tc.tile_pool(name="ps_m", bufs=1, space="PSUM")),
        "ps_epi": ctx.enter_context(tc.tile_pool(name="ps_epi", bufs=2, space="PSUM")),
    }


def _emit_body(pools, nc, tc, dram, mybir):
    """Emit one full forward pass for one core."""
    f32 = mybir.dt.float32
    bf16 = mybir.dt.bfloat16
    AF = mybir.ActivationFunctionType
    OP = mybir.AluOpType

    consts = pools["consts"]
    xtp = pools["xtp"]
    xnp = pools["xnp"]
    bigp = pools["bigp"]
    grids = pools["grids"]
    scratch = pools["scratch"]
    ps_lg = pools["ps_lg"]
    ps_m = pools["ps_m"]
    ps_epi = pools["ps_epi"]

    # [512, X] dram tensors load as one [128, 4, X] tile each (one DMA).
    def load4(src, cols, dtype, nm):
        t = consts.tile([P, DT, cols], dtype, tag=nm, name=nm)
        nc.sync.dma_start(
            out=t[:], in_=src[:, :].rearrange("(a p) c -> p a c", p=P))
        return t

    # ---- head: minimal-latency first work --------------------------------
    # DMA order tuned so the first matmul can issue ~2us in: aw0 + the first
    # feature-tile of xt0 arrive first, then the rest streams while PE works.
    aw = []
    for k in range(DT):
        t = consts.tile([P, D], bf16, tag=f"aw{k}", name=f"aw{k}")
        aw.append(t)
    nc.sync.dma_start(out=aw[0][:], in_=dram["attn_w"][0:P, :])
    xt0 = xtp.tile([P, DT, JT], bf16, tag="xt", name="xt0")
    nc.sync.dma_start(out=xt0[:, 0:1], in_=dram["xT"][0:P, 0:JT].rearrange(
        "(a p) c -> p a c", p=P))
    nc.sync.dma_start(
        out=xt0[:, 1:DT],
        in_=dram["xT"][P:D, 0:JT].rearrange("(a p) c -> p a c", p=P))
    for k in range(1, DT):
        nc.sync.dma_start(out=aw[k][:], in_=dram["attn_w"][k * P:(k + 1) * P, :])
    # prefetch the next two stream tiles ahead of everything non-urgent so
    # PE never waits on the serialized DMA queue
    xt_pre = {}
    for j in (1, 2):
        t = xtp.tile([P, DT, JT], bf16, tag="xt", name=f"xt{j}")
        nc.sync.dma_start(
            out=t[:],
            in_=dram["xT"][:, j * JT:(j + 1) * JT].rearrange(
                "(a p) c -> p a c", p=P))
        xt_pre[j] = t
    G = consts.tile([P, 2 * P], bf16, tag="G", name="G")
    nc.sync.dma_start(out=G[:], in_=dram["G"][:])
    xn0 = xnp.tile([P, 4, D], bf16, tag="xn", name="xn0")
    nc.sync.dma_start(
        out=xn0[:], in_=dram["xn"][0:JT, :].rearrange("(t p) c -> p t c", p=P))

    # grids for segsum results (combined across d-tiles)
    Zc = grids.tile([P, DT, NCH], f32, tag="Zc", name="Zc")[:]
    Nc = grids.tile([P, DT, NCH], f32, tag="Nc", name="Nc")[:]
    m_ps = ps_m.tile([P, D], f32, tag="m_ps", name="m_ps")

    ones11 = consts.tile([1, 1], f32, tag="ones11", name="ones11")
    nc.vector.memset(ones11[:], 1.0)
    ones1p = consts.tile([1, P], f32, tag="ones1p", name="ones1p")
    nc.vector.memset(ones1p[:], 1.0)

    def emit_front():
        # front work that depends only on host-prepped firsts/lasts; DMAs
        # deferred into the stream so they don't delay the xt/xn tiles
        fp4 = load4(dram["fpad"], NCH + 1, f32, "fp4")
        lp4 = load4(dram["lpad"], NCH + 1, f32, "lp4")
        rw14 = load4(dram["router_w1"], HID, f32, "rw14")
        rw1 = [rw14[:, k] for k in range(DT)]
        rb1 = consts.tile([1, HID], f32, tag="rb1", name="rb1")
        nc.sync.dma_start(out=rb1[:], in_=dram["router_b1"][:])
        rw2 = consts.tile([HID, NEXP], f32, tag="rw2", name="rw2")
        nc.sync.dma_start(out=rw2[:], in_=dram["router_w2"][:])
        rb2 = consts.tile([1, NEXP], f32, tag="rb2", name="rb2")
        nc.sync.dma_start(out=rb2[:], in_=dram["router_b2"][:])

        # boundary terms on the (otherwise idle at this point) GpSimd engine
        u = grids.tile([P, DT, NCH], f32, tag="u", name="u")
        nc.vector.tensor_tensor(out=u[:], in0=lp4[:, :, 0:NCH],
                                in1=lp4[:, :, 1:NCH + 1], op=OP.subtract)
        v = grids.tile([P, DT, NCH], f32, tag="v", name="v")
        nc.vector.tensor_tensor(out=v[:], in0=fp4[:, :, 1:NCH + 1],
                                in1=fp4[:, :, 0:NCH], op=OP.subtract)
        xfs = grids.tile([P, DT], f32, tag="xfs", name="xfs")
        nc.vector.reduce_sum(out=xfs[:], in_=fp4[:, :, 0:NCH],
                             axis=mybir.AxisListType.X)

        # router MLP + softmax + broadcast of r (independent of the stream)
        xf = grids.tile([P, DT], f32, tag="xf", name="xf")
        nc.scalar.mul(xf[:], xfs[:], 1.0 / NCH)
        ps_h = ps_epi.tile([P, 1], f32, tag="epi", name="epi")
        for k in range(DT):
            nc.tensor.matmul(ps_h[:], rw1[k][:], xf[:, k:k + 1],
                             start=(k == 0), stop=False)
        nc.tensor.matmul(ps_h[:], rb1[:], ones11[:], start=False, stop=True)
        hsb = grids.tile([P, 1], f32, tag="hsb", name="hsb")
        nc.scalar.activation(out=hsb[:], in_=ps_h[:], func=AF.Relu)
        ps_r = ps_epi.tile([1, NEXP], f32, tag="epi", name="epi")
        nc.tensor.matmul(ps_r[:], hsb[:], rw2[:], start=True, stop=False)
        nc.tensor.matmul(ps_r[:], ones11[:], rb2[:], start=False, stop=True)
        rmax = grids.tile([1, 1], f32, tag="rmax", name="rmax")
        nc.vector.reduce_max(out=rmax[:], in_=ps_r[:],
                             axis=mybir.AxisListType.X)
        nrmax = grids.tile([1, 1], f32, tag="nrmax", name="nrmax")
        nc.vector.tensor_scalar_mul(nrmax[:], rmax[:], -1.0)
        er = grids.tile([1, NEXP], f32, tag="er", name="er")
        nc.scalar.activation(out=er[:], in_=ps_r[:], func=AF.Exp, bias=nrmax[:])
        rsum = grids.tile([1, 1], f32, tag="rsum", name="rsum")
        nc.vector.reduce_sum(out=rsum[:], in_=er[:], axis=mybir.AxisListType.X)
        rrec = grids.tile([1, 1], f32, tag="rrec", name="rrec")
        nc.vector.reciprocal(rrec[:], rsum[:])
        rvec = grids.tile([1, NEXP], f32, tag="rvec", name="rvec")
        nc.vector.tensor_scalar_mul(rvec[:], er[:], rrec[:])
        ps_b = ps_epi.tile([P, NEXP], f32, tag="epi", name="epi")
        nc.tensor.matmul(ps_b[:], ones1p[:], rvec[:], start=True, stop=True)
        rb = grids.tile([P, NEXP], f32, tag="rb", name="rb")
        nc.scalar.copy(rb[:], ps_b[:])
        return u, v, rb

    PIECES = [1, 1, 2, 2, 2, 2, 2, 2, 1, 1]  # small pieces: segsum work starts
    assert sum(PIECES) == NJ                 # early and ends with a tiny tail

    NCH_MAX = max(PIECES) * JT // C

    def make_tree_gen(Ep, Pp, ch0, nch):
        """Segsum per tensor (all 4 d-tiles in one op): four bf16 pair-add
        levels (DVE 2x mode) then a cheap fp32 reduce of the last 4 — the
        f32 reduce gets no DVE fast mode, so keep it small."""
        for tile4, grid in ((Ep, Zc), (Pp, Nc)):
            view = tile4[:, :, 0:nch * C].rearrange("p a (n c) -> p a n c", c=C)
            s1 = scratch.tile([P, DT, NCH_MAX, C // 2], bf16, tag="s1",
                              name="s1", bufs=2)[:, :, 0:nch]
            nc.vector.tensor_tensor(
                out=s1, in0=view[:, :, :, 0:32], in1=view[:, :, :, 32:64],
                op=OP.add)
            yield
            s2 = scratch.tile([P, DT, NCH_MAX, C // 4], bf16, tag="s2",
                              name="s2", bufs=2)[:, :, 0:nch]
            nc.vector.tensor_tensor(
                out=s2, in0=s1[:, :, :, 0:16], in1=s1[:, :, :, 16:32],
                op=OP.add)
            yield
            s3 = scratch.tile([P, DT, NCH_MAX, C // 8], bf16, tag="s3",
                              name="s3", bufs=2)[:, :, 0:nch]
            nc.vector.tensor_tensor(
                out=s3, in0=s2[:, :, :, 0:8], in1=s2[:, :, :, 8:16],
                op=OP.add)
            yield
            s4 = scratch.tile([P, DT, NCH_MAX, C // 16], bf16, tag="s4",
                              name="s4", bufs=2)[:, :, 0:nch]
            nc.vector.tensor_tensor(
                out=s4, in0=s3[:, :, :, 0:4], in1=s3[:, :, :, 4:8],
                op=OP.add)
            yield
            nc.vector.reduce_sum(
                out=grid[:, :, ch0:ch0 + nch], in_=s4,
                axis=mybir.AxisListType.X)
            yield
            if tile4 is Ep:
                # 1/Z for this piece now, in stream slack, so the output
                # mixes at the tail are only two ops per range
                nc.vector.reciprocal(rz[:, :, ch0:ch0 + nch],
                                     Zc[:, :, ch0:ch0 + nch])
                yield

    mconv_loads = {}

    def emit_mconv_load(step):
        # conv weights / bias / ident DMAs, one per stream tile so they
        # never delay an xt/xn tile in the serialized DMA queue
        if step < 3:
            w4 = load4(dram[f"w{step}T"], D, bf16, f"w{step}T4")
            mconv_loads[step] = [w4[:, k] for k in range(DT)]
        else:
            cbr = consts.tile([1, D], f32, tag="cbr", name="cbr")
            nc.sync.dma_start(out=cbr[:], in_=dram["conv_b_row"][:])
            ident = consts.tile([P, P], f32, tag="ident", name="ident")
            nc.sync.dma_start(out=ident[:], in_=dram["ident"][:])
            mconv_loads["cbr"] = cbr
            mconv_loads["ident"] = ident

    def emit_mconv():
        # ------- epilogue part A: m transpose + conv expert
        wT = mconv_loads
        cbr = mconv_loads["cbr"]
        ident = mconv_loads["ident"]
        u, v, rb = front_out

        # m: PSUM [128 chunks, 512 d] -> SBUF f32 (scaled 1/64) -> transpose
        m_nat = grids.tile([P, D], f32, tag="m_nat", name="m_nat")
        nc.scalar.mul(m_nat[:], m_ps[:], 1.0 / C)
        mT = grids.tile([P, DT, NCH], f32, tag="mT", name="mT")
        for k in range(DT):
            pst = ps_epi.tile([P, P], f32, tag="epi", name="epi")
            nc.tensor.transpose(pst[:], m_nat[:, k * P:(k + 1) * P], ident[:])
            nc.scalar.copy(mT[:, k], pst[:])
        mTb = grids.tile([P, DT, NCH], bf16, tag="mTb", name="mTb")
        nc.scalar.copy(mTb[:], mT[:])

        # a = m + u/64, c = m + v/64  (bf16 for matmul; on GpSimd)
        aTb = grids.tile([P, DT, NCH], bf16, tag="aTb", name="aTb")
        nc.vector.scalar_tensor_tensor(
            out=aTb[:], in0=u[:], scalar=1.0 / C, in1=mT[:],
            op0=OP.mult, op1=OP.add)
        cTb = grids.tile([P, DT, NCH], bf16, tag="cTb", name="cTb")
        nc.vector.scalar_tensor_tensor(
            out=cTb[:], in0=v[:], scalar=1.0 / C, in1=mT[:],
            op0=OP.mult, op1=OP.add)

        # conv expert: 12 matmuls + bias matmul, then copy to SBUF
        convT = grids.tile([P, DT, NCH], f32, tag="convT", name="convT")
        for o in range(DT):
            ps = ps_epi.tile([P, NCH], f32, tag="epi", name="epi")
            first = True
            for w, rhs4 in ((0, aTb), (1, mTb), (2, cTb)):
                for k in range(DT):
                    nc.tensor.matmul(
                        ps[:], wT[w][k][:, o * P:(o + 1) * P], rhs4[:, k],
                        start=first, stop=False)
                    first = False
            nc.tensor.matmul(
                ps[:], cbr[:, o * P:(o + 1) * P], ones1p[:],
                start=False, stop=True)
            nc.scalar.copy(convT[:, o], ps[:])

        # r0 * m term of the mix (ready early; ACT per-partition scale),
        # then fold in r2*conv so each output range needs only 2 DVE ops
        tmp = grids.tile([P, DT, NCH], f32, tag="tmp", name="tmp")
        nc.scalar.mul(tmp[:], mT[:], rb[:, 0:1])
        W = grids.tile([P, DT, NCH], f32, tag="W", name="W")
        nc.vector.scalar_tensor_tensor(
            out=W[:], in0=convT[:], scalar=rb[:, 2:3], in1=tmp[:],
            op0=OP.mult, op1=OP.add)
        return W

    rz = grids.tile([P, DT, NCH], f32, tag="rz", name="rz")
    attnT = grids.tile([P, DT, NCH], f32, tag="attnT", name="attnT")
    y4 = grids.tile([P, DT, NCH], f32, tag="y4", name="y4")

    def emit_mix(c0, c1):
        """Mix experts and DMA out chunks [c0:c1) (grids + rz final):
        attn = N*(1/Z), y = r1*attn + (r0*m + r2*conv), then stream out."""
        W = mconv_out
        u, v, rb = front_out
        sl = slice(c0, c1)
        nc.vector.tensor_tensor(out=attnT[:, :, sl], in0=Nc[:, :, sl],
                                in1=rz[:, :, sl], op=OP.mult)
        nc.vector.scalar_tensor_tensor(
            out=y4[:, :, sl], in0=attnT[:, :, sl], scalar=rb[:, 1:2],
            in1=W[:, :, sl], op0=OP.mult, op1=OP.add)
        nc.sync.dma_start(
            out=dram["y"][:, :].rearrange("(a p) n -> p a n", p=P)[:, :, sl],
            in_=y4[:, :, sl])

    # ---------------- main streaming phase ----------------
    # xn (natural layout) and the chunk-sum matmuls for m start at j==2 so
    # PE's in-order stream never waits on the xn DMAs during the head; they
    # still finish by j==12, in time for the conv epilogue at j==13.
    pending = None
    jbase = 0
    jn = 0
    xn_cur = xn0
    xn_nxt = None
    xn_tile_idx = 0
    front_out = None
    mconv_out = None

    def gmm_until(limit):
        nonlocal jn, xn_cur, xn_nxt, xn_tile_idx
        while jn < min(limit, 4 * NJ):
            t = jn // 4
            if t != xn_tile_idx:
                xn_cur = xn_nxt
                xn_tile_idx = t
                if t + 1 < NJ:
                    xn_nxt = prefetch_xn(t + 1)
            nc.tensor.matmul(
                m_ps[:], G[:, P - 2 * jn:2 * P - 2 * jn], xn_cur[:, jn % 4],
                start=(jn == 0), stop=(jn == 4 * NJ - 1),
                skip_group_check=True)
            jn += 1

    def prefetch_xn(t):
        tile = xnp.tile([P, 4, D], bf16, tag="xn", name="xn")
        nc.sync.dma_start(
            out=tile[:],
            in_=dram["xn"][t * JT:(t + 1) * JT, :].rearrange(
                "(t p) c -> p t c", p=P))
        return tile

    def fetch_xt(j):
        if j in xt_pre:
            return xt_pre.pop(j)
        tile = xtp.tile([P, DT, JT], bf16, tag="xt", name="xt")
        nc.sync.dma_start(
            out=tile[:],
            in_=dram["xT"][:, j * JT:(j + 1) * JT].rearrange(
                "(a p) c -> p a c", p=P))
        return tile

    def last_tile_otile(xt, Ep, Pp, o, ps):
        """j==15: per-otile exp + multiply + segsum so the drain pipelines
        behind each PSUM bank instead of waiting for the whole tile."""
        nc.scalar.activation(out=Ep[:, o, 0:JT], in_=ps[:], func=AF.Exp)
        nc.vector.tensor_tensor(out=Pp[:, o, 0:JT], in0=xt[:, o],
                                in1=Ep[:, o, 0:JT], op=OP.mult)
        ch0 = 15 * JT // C
        for tpfx, src, grid in (("e", Ep, Zc), ("p", Pp, Nc)):
            view = src[:, o, 0:JT].rearrange("p (n c) -> p n c", c=C)
            l1 = scratch.tile([P, JT // C, C // 2], bf16, tag=f"l1{tpfx}{o}",
                              name=f"l1{tpfx}{o}")
            nc.vector.tensor_tensor(out=l1[:], in0=view[:, :, 0:32],
                                    in1=view[:, :, 32:64], op=OP.add)
            l2 = scratch.tile([P, JT // C, C // 4], bf16, tag=f"l2{tpfx}{o}",
                              name=f"l2{tpfx}{o}")
            nc.vector.tensor_tensor(out=l2[:], in0=l1[:, :, 0:16],
                                    in1=l1[:, :, 16:32], op=OP.add)
            nc.vector.reduce_sum(
                out=grid[:, o, ch0:ch0 + JT // C],
                in_=l2[:], axis=mybir.AxisListType.X)
            if tpfx == "e":
                nc.vector.reciprocal(rz[:, o, ch0:ch0 + JT // C],
                                     Zc[:, o, ch0:ch0 + JT // C])

    for pc in range(len(PIECES)):
        PJp = PIECES[pc]
        Ep = bigp.tile([P, DT, max(PIECES) * JT], bf16, tag="Ep", name="Ep",
                       bufs=2)
        Pp = bigp.tile([P, DT, max(PIECES) * JT], bf16, tag="Pp", name="Pp",
                       bufs=2)
        for jj in range(PJp):
            j = jbase + jj
            off = jj * JT
            xt = xt0 if j == 0 else fetch_xt(j)
            if j + 2 < NJ and j + 2 > 2:
                xt_pre[j + 2] = fetch_xt(j + 2)
            if j == 1:
                xn_nxt = prefetch_xn(1)
            if j == NJ - 1 and pending is not None:
                # drain the previous piece fully BEFORE the last tile's
                # per-otile trees so its reduce isn't stuck behind them in
                # DVE's in-order queue (the final mix needs it)
                for _ in pending:
                    pass
                pending = None
            if j == 0:
                # k-outer order so the first matmul only needs aw0 + the
                # first feature-tile of xt0 (shortest DMA critical path).
                ps4 = [ps_lg.tile([P, JT], f32, tag="lg", name="lg")
                       for _ in range(DT)]
                for k in range(DT):
                    for o in range(DT):
                        nc.tensor.matmul(
                            ps4[o][:], aw[k][:, o * P:(o + 1) * P], xt[:, k],
                            start=(k == 0), stop=(k == DT - 1))
                for o in range(DT):
                    nc.scalar.activation(
                        out=Ep[:, o, off:off + JT], in_=ps4[o][:], func=AF.Exp)
                nc.vector.tensor_tensor(
                    out=Pp[:, :, off:off + JT], in0=xt[:],
                    in1=Ep[:, :, off:off + JT], op=OP.mult)
            else:
                for o in range(DT):
                    ps = ps_lg.tile([P, JT], f32, tag="lg", name="lg")
                    for k in range(DT):
                        nc.tensor.matmul(
                            ps[:], aw[k][:, o * P:(o + 1) * P], xt[:, k],
                            start=(k == 0), stop=(k == DT - 1))
                    if j == NJ - 1:
                        last_tile_otile(xt, Ep, Pp, o, ps)
                    else:
                        nc.scalar.activation(
                            out=Ep[:, o, off:off + JT], in_=ps[:], func=AF.Exp)
                if j != NJ - 1:
                    nc.vector.tensor_tensor(
                        out=Pp[:, :, off:off + JT], in0=xt[:],
                        in1=Ep[:, :, off:off + JT], op=OP.mult)
            if j >= 2:
                gmm_until(6 * (j - 1))
            # interleave previous piece's segsum ops (6 per pos-tile)
            if pending is not None:
                for _ in range(6):
                    if next(pending, "done") == "done":
                        pending = None
                        break
            if j == 4:
                front_out = emit_front()
            if 7 <= j <= 10:
                emit_mconv_load(j - 7)
            if j == 13:
                mconv_out = emit_mconv()
            if j == 14:
                emit_mix(0, 96)       # pieces 0-6 drained long ago
            if j == 15:
                emit_mix(96, 112)     # piece 7 drained by early j15
        if pending is not None:
            for _ in pending:
                pass
        if pc < len(PIECES) - 1:
            pending = make_tree_gen(Ep, Pp, jbase * JT // C, PJp * JT // C)
        jbase += PJp

    # ------- final mix (last piece's per-otile trees already emitted)
    emit_mix(112, NCH)


def _build(loop_iters=None):
    import concourse.bass as bass
    from concourse import bacc
    import concourse.mybir as mybir
    import concourse.tile as tile

    f32 = mybir.dt.float32
    bf16 = mybir.dt.bfloat16

    nc = bacc.Bacc(None, target_bir_lowering=False)
    dram = {
        "xT": nc.dram_tensor("xT", [D, S], bf16, kind="ExternalInput"),
        "xn": nc.dram_tensor("xn", [S, D], bf16, kind="ExternalInput"),
        "attn_w": nc.dram_tensor("attn_w", [D, D], bf16, kind="ExternalInput"),
        "w0T": nc.dram_tensor("w0T", [D, D], bf16, kind="ExternalInput"),
        "w1T": nc.dram_tensor("w1T", [D, D], bf16, kind="ExternalInput"),
        "w2T": nc.dram_tensor("w2T", [D, D], bf16, kind="ExternalInput"),
        "fpad": nc.dram_tensor("fpad", [D, NCH + 1], f32, kind="ExternalInput"),
        "lpad": nc.dram_tensor("lpad", [D, NCH + 1], f32, kind="ExternalInput"),
        "router_w1": nc.dram_tensor("router_w1", [D, HID], f32, kind="ExternalInput"),
        "router_b1": nc.dram_tensor("router_b1", [1, HID], f32, kind="ExternalInput"),
        "router_w2": nc.dram_tensor("router_w2", [HID, NEXP], f32, kind="ExternalInput"),
        "router_b2": nc.dram_tensor("router_b2", [1, NEXP], f32, kind="ExternalInput"),
        "conv_b_row": nc.dram_tensor("conv_b_row", [1, D], f32, kind="ExternalInput"),
        "G": nc.dram_tensor("G", [P, 2 * P], bf16, kind="ExternalInput"),
        "ident": nc.dram_tensor("ident", [P, P], f32, kind="ExternalInput"),
        "y": nc.dram_tensor("y", [D, NCH], f32, kind="ExternalOutput"),
    }
    from contextlib import ExitStack
    with tile.TileContext(nc) as tc:
        with ExitStack() as ctx:
            pools = _make_pools(ctx, tc)
            # Preload the activation-function table once, outside the loop
            # body (otherwise LoadActFuncSet costs ~1.3us on every iteration).
            warm = pools["consts"].tile([1, 1], mybir.dt.float32, tag="warm",
                                        name="warm")
            nc.vector.memset(warm[:], 0.0)
            warm2 = pools["consts"].tile([1, 1], mybir.dt.float32, tag="warm2",
                                         name="warm2")
            nc.scalar.activation(out=warm2[:], in_=warm[:],
                                 func=mybir.ActivationFunctionType.Exp)
            if loop_iters is None:
                _emit_body(pools, nc, tc, dram, mybir)
            else:
                ET = mybir.EngineType
                with tc.For_i(0, loop_iters, 1, staggered_reset=True,
                              hint_engines=(ET.PE, ET.DVE, ET.Activation,
                                            ET.SP)):
                    _emit_body(pools, nc, tc, dram, mybir)
    nc.finalize()
    return nc


def _host_prep(inputs):
    """Build per-core input maps from full inputs."""
    x = np.asarray(inputs["x"], dtype=np.float32)
    attn_w = np.asarray(inputs["attn_w"], dtype=np.float32)
    conv_w = np.asarray(inputs["conv_w"], dtype=np.float32)
    conv_b = np.asarray(inputs["conv_b"], dtype=np.float32)
    rw1 = np.asarray(inputs["router_w1"], dtype=np.float32)
    rb1 = np.asarray(inputs["router_b1"], dtype=np.float32)
    rw2 = np.asarray(inputs["router_w2"], dtype=np.float32)
    rb2 = np.asarray(inputs["router_b2"], dtype=np.float32)

    aw_bf = np.ascontiguousarray(attn_w).astype(BF16)
    w0T = np.ascontiguousarray(conv_w[:, :, 0].T).astype(BF16)
    w1T = np.ascontiguousarray(conv_w[:, :, 1].T).astype(BF16)
    w2T = np.ascontiguousarray(conv_w[:, :, 2].T).astype(BF16)
    G = np.zeros((P, 2 * P), BF16)
    G[0:C, P] = 1.0
    G[C:P, P + 1] = 1.0
    ident = np.eye(P, dtype=np.float32)
    rb1_2d = rb1.reshape(1, HID)
    rb2_2d = rb2.reshape(1, NEXP)
    cb_row = conv_b.reshape(1, D)

    in_maps = []
    for b in range(B):
        xb = x[b]
        F = xb[0::C]            # [NCH, D]
        L = xb[C - 1::C]
        fpad = np.zeros((D, NCH + 1), np.float32)
        fpad[:, 0:NCH] = F.T
        lpad = np.zeros((D, NCH + 1), np.float32)
        lpad[:, 1:NCH + 1] = L.T
        in_maps.append({
            "xT": np.ascontiguousarray(xb.T).astype(BF16),
            "xn": xb.astype(BF16),
            "attn_w": aw_bf,
            "w0T": w0T, "w1T": w1T, "w2T": w2T,
            "fpad": fpad, "lpad": lpad,
            "router_w1": rw1, "router_b1": rb1_2d,
            "router_w2": rw2, "router_b2": rb2_2d,
            "conv_b_row": cb_row, "G": G, "ident": ident,
        })
    return in_maps


def kernel(**inputs):
    from concourse.bass_utils import run_bass_kernel_spmd

    if "nc" not in _CACHE:
        _CACHE["nc"] = _build()
    nc = _CACHE["nc"]
    in_maps = _host_prep(inputs)
    res = run_bass_kernel_spmd(nc, in_maps, list(range(N_CORES)))
    out = np.stack([np.ascontiguousarray(res.results[b]["y"].T)
                    for b in range(B)])
    return out.astype(np.float32)


if __name__ == "__main__":
    rng = np.random.default_rng(0)
    fake = {
        "x": rng.standard_normal((B, S, D), dtype=np.float32),
        "attn_w": rng.standard_normal((D, D), dtype=np.float32) / np.sqrt(D),
        "attn_b": np.zeros(D, np.float32),
        "conv_w": rng.standard_normal((D, D, 3), dtype=np.float32) / np.sqrt(3 * D),
        "conv_b": np.zeros(D, np.float32),
        "router_w1": rng.standard_normal((D, HID), dtype=np.float32) / np.sqrt(D),
        "router_b1": np.zeros(HID, np.float32),
        "router_w2": rng.standard_normal((HID, NEXP), dtype=np.float32) / np.sqrt(HID),
        "router_b2": np.zeros(NEXP, np.float32),
    }
    y = kernel(**fake)
    print("kernel out", y.shape, y.dtype, np.abs(y).max())
